# revision 6
# baseline (speedup 1.0000x reference)
"""Trainium2 Bass kernel for nn_Attention_62672162783289.

Dense transformer attention block: LayerNorm -> fused QKV -> per-head scaled
dot-product attention with gathered relative-position bias -> output proj.

Sharding: data-parallel over batch B=16 across 8 NeuronCores (2 batches/core).
No collectives needed; outputs are concatenated on the host.

Device-side design (per core, all matmuls bf16 with fp32 PSUM accumulation):
  - x is host-transposed to feature-major [2, 768, 1024] so LayerNorm stats are
    column sums computed with ones-matmuls; ln_w/ln_b are folded into the QKV
    weights on the host, so the device only applies (x - mu) * rstd.
  - rstd = exp(-0.5*ln(var+eps)) on ScalarE (DVE reciprocal is 8 cyc/elem).
  - Q,K are produced feature-major in head-pair tiles (q_2p|q_2p+1 /
    k_2p|k_2p+1 on partition halves) so score matmuls for the two heads of a
    pair run concurrently in different PE row groups (K=64 each).
  - Scores are computed transposed, S^T[m,n], so softmax normalization is a
    column sum obtained for free from an extra ones-column in the AV lhsT.
  - Softmax uses no max subtraction (scores are in [-2, 2] for this problem);
    the additive position bias is applied multiplicatively: exp(s+b) =
    exp(s)*exp(b) with exp(b) precomputed on the host (it is input-independent).
  - V is produced token-major [m, (h,65)] with an interleaved ones column so it
    can be used directly as the AV stationary operand.
  - AV output arrives transposed [(d|ones), n]; after dividing by the ones-row
    (denominator, bcast via DRAM bounce + exp(-ln(d))), it is exactly the
    proj contraction layout.
"""
import os
import numpy as np
import ml_dtypes

import concourse.bass as bass
import concourse.tile as tile
from concourse import bacc, mybir

bf16 = mybir.dt.bfloat16
f32 = mybir.dt.float32
FP = mybir.ActivationFunctionType

B, RES, DIM, H, KD = 16, 32, 768, 12, 64
N = RES * RES            # 1024 tokens
DH = KD * H              # 768
NCORES = 8
BL = B // NCORES         # 2 batches per core
SCALE = KD ** -0.5
LN_EPS = 1e-5
CHUNKS = DIM // 128      # 6 contraction chunks
PAIRS = H // 2           # 6 head pairs
MT = N // 128            # 8 m-tiles
TT = N // 128            # 8 token tiles


def _bcast_ap(dram_ap, nparts):
    """Partition-step-0 broadcast AP over a DRAM row region."""
    return bass.AP(tensor=dram_ap.tensor, offset=dram_ap.offset,
                   ap=[[0, nparts]] + dram_ap.ap[1:])


def build_program(reps=1):
    nc = bacc.Bacc("TRN2", target_bir_lowering=False, debug=False,
                   num_devices=NCORES)

    x_d = nc.dram_tensor("x", [BL, DIM, N], bf16, kind="ExternalInput").ap()
    wqk_d = nc.dram_tensor("wqk", [DIM, 2 * DH], bf16, kind="ExternalInput").ap()
    qkb_d = nc.dram_tensor("qkb", [H, 128, 1], f32, kind="ExternalInput").ap()
    wv_d = nc.dram_tensor("wv", [DIM, DH], bf16, kind="ExternalInput").ap()
    vb_d = nc.dram_tensor("vb", [1, H * 65], f32, kind="ExternalInput").ap()
    pw_d = nc.dram_tensor("pw", [DH, DIM], bf16, kind="ExternalInput").ap()
    pb_d = nc.dram_tensor("pb", [1, DIM], f32, kind="ExternalInput").ap()
    eb_d = nc.dram_tensor("eb", [H, MT, 128, N], bf16, kind="ExternalInput").ap()
    out_d = nc.dram_tensor("out", [BL, N, DIM], f32, kind="ExternalOutput").ap()

    den_scr = nc.dram_tensor("den_scr", [PAIRS * BL, 4 * 512], f32).ap()
    rec_scr = nc.dram_tensor("rec_scr", [PAIRS * BL, 4 * 512], bf16).ap()
    ab_scr = nc.dram_tensor("ab_scr", [BL, N], bf16).ap()
    bb_scr = nc.dram_tensor("bb_scr", [BL, N], bf16).ap()

    with tile.TileContext(nc) as tc:
        with (
            tc.tile_pool(name="persist", bufs=1) as persist,
            tc.tile_pool(name="qkvout", bufs=1) as qkvout,
        ):
            # ---- persistent weights / constants
            pw_sb = []
            for p in range(PAIRS):
                t = persist.tile([128, DIM], bf16, tag=f"pw{p}")
                nc.sync.dma_start(t[:], pw_d[p * 128:(p + 1) * 128, :])
                pw_sb.append(t)
            projbB = persist.tile([128, DIM], f32, tag="projbB")
            nc.gpsimd.dma_start(projbB[:], _bcast_ap(pb_d[0:1, :], 128))

            for rep in range(reps):
                _emit_body(nc, tc, persist, qkvout, rep,
                           x_d, wqk_d, qkb_d, wv_d, vb_d, eb_d, out_d,
                           den_scr, rec_scr, ab_scr, bb_scr, pw_sb, projbB)

    nc.compile()
    return nc


def _emit_body(nc, tc, persist, qkvout, rep,
               x_d, wqk_d, qkb_d, wv_d, vb_d, eb_d, out_d,
               den_scr, rec_scr, ab_scr, bb_scr, pw_sb, projbB):
    # tags are shared across reps: WAR deps serialize reps (correct; used for timing)
    r = ""
    qk_sb = [[None] * H for _ in range(BL)]  # 12 f-tiles: even=QPAIR, odd=KPAIR   # [b][ft] -> [128, N] bf16
    v_sb = [[None] * TT for _ in range(BL)]   # [b][tt] -> [128, H*65] bf16
    projIn = [[None] * PAIRS for _ in range(BL)]

    # ======== Phase 1: LayerNorm + QKV (both batches) ========
    with (
        tc.tile_pool(name="wqkp", bufs=1) as wqkp,
        tc.tile_pool(name="xp", bufs=7) as xp,
        tc.tile_pool(name="sqp", bufs=2) as sqp,
        tc.tile_pool(name="rowp", bufs=1) as rowp,
        tc.tile_pool(name="lnbc", bufs=2) as lnbc,
        tc.tile_pool(name="xhp", bufs=12) as xhp,
        tc.tile_pool(name="tmpp", bufs=2) as tmpp,
        tc.tile_pool(name="lnps", bufs=2, space="PSUM") as lnps,
        tc.tile_pool(name="qkps", bufs=2, space="PSUM") as qkps,
        tc.tile_pool(name="vps", bufs=2, space="PSUM") as vps,
    ):
        wqk_sb, wv_sb, qkb_sb = [], [], []
        for c in range(CHUNKS):
            t = wqkp.tile([128, 2 * DH], bf16, tag=f"wqk{c}")
            nc.sync.dma_start(t[:], wqk_d[c * 128:(c + 1) * 128, :])
            wqk_sb.append(t)
            t = wqkp.tile([128, DH], bf16, tag=f"wv{c}")
            nc.sync.dma_start(t[:], wv_d[c * 128:(c + 1) * 128, :])
            wv_sb.append(t)
        for ft in range(H):
            t = wqkp.tile([128, 1], f32, tag=f"qkb{ft}")
            nc.sync.dma_start(t[:], qkb_d[ft])
            qkb_sb.append(t)
        vbB = wqkp.tile([128, H * 65], f32, tag="vbB")
        nc.gpsimd.dma_start(vbB[:], _bcast_ap(vb_d[0:1, :], 128))
        ones_c = wqkp.tile([128, 1], bf16, tag="ones")
        nc.vector.memset(ones_c[:], 1.0)
        eps_t = wqkp.tile([128, 1], f32, tag="eps")
        nc.vector.memset(eps_t[:], LN_EPS)

        for b in range(BL):
            # -- load x chunks (feature-major)
            xc = []
            for c in range(CHUNKS):
                t = xp.tile([128, N], bf16)
                nc.sync.dma_start(t[:], x_d[b, c * 128:(c + 1) * 128, :])
                xc.append(t)

            # -- stats: column sums of x and x^2 via ones-matmuls
            mrow = rowp.tile([1, N], f32, tag="mrow")
            srow = rowp.tile([1, N], f32, tag="srow")
            for tb in range(2):
                mean_ps = lnps.tile([1, 512], f32, tag="lnstat")
                sq_ps = lnps.tile([1, 512], f32, tag="lnstat")
                for c in range(CHUNKS):
                    sl = xc[c][:, tb * 512:(tb + 1) * 512]
                    sq = sqp.tile([128, 512], bf16)
                    nc.scalar.activation(sq[:], sl, FP.Square)
                    nc.tensor.matmul(mean_ps[:], ones_c[:], sl,
                                     start=(c == 0), stop=(c == CHUNKS - 1))
                    nc.tensor.matmul(sq_ps[:], ones_c[:], sq[:],
                                     start=(c == 0), stop=(c == CHUNKS - 1))
                nc.scalar.copy(mrow[0:1, tb * 512:(tb + 1) * 512], mean_ps[:])
                nc.scalar.copy(srow[0:1, tb * 512:(tb + 1) * 512], sq_ps[:])

            # -- row math: alpha = rstd, beta = -mu*rstd (1 lane, in-place)
            nc.vector.tensor_scalar(mrow[:], mrow[:], 1.0 / DIM, None,
                                    mybir.AluOpType.mult)  # mrow = mu
            mu2 = rowp.tile([1, N], f32, tag="mu2")
            nc.vector.tensor_mul(mu2[:], mrow[:], mrow[:])
            nc.vector.scalar_tensor_tensor(srow[:], srow[:], 1.0 / DIM, mu2[:],
                                           mybir.AluOpType.mult,
                                           mybir.AluOpType.subtract)  # srow = var
            nc.scalar.activation(srow[:], srow[:], FP.Ln, bias=eps_t[0:1, :])
            arow = rowp.tile([1, N], bf16, tag="arow")
            nc.scalar.activation(arow[:], srow[:], FP.Exp, scale=-0.5)
            brow = rowp.tile([1, N], bf16, tag="brow")
            nc.vector.scalar_tensor_tensor(brow[:], mrow[:], -1.0, arow[:],
                                           mybir.AluOpType.mult,
                                           mybir.AluOpType.mult)
            nc.sync.dma_start(ab_scr[b:b + 1, :], arow[:])
            nc.sync.dma_start(bb_scr[b:b + 1, :], brow[:])
            alphaB = lnbc.tile([128, N], bf16, tag="alphaB")
            nc.gpsimd.dma_start(alphaB[:], _bcast_ap(ab_scr[b:b + 1, :], 128))
            betaB = lnbc.tile([128, N], bf16, tag="betaB")
            nc.gpsimd.dma_start(betaB[:], _bcast_ap(bb_scr[b:b + 1, :], 128))

            # -- xhat = x*alpha + beta (bf16)
            xh = []
            for c in range(CHUNKS):
                t0 = tmpp.tile([128, N], bf16)
                nc.vector.tensor_mul(t0[:], xc[c][:], alphaB[:])
                t = xhp.tile([128, N], bf16)
                nc.vector.tensor_add(t[:], t0[:], betaB[:])
                xh.append(t)

            # -- Q,K feature-major head-pair tiles (12 f-tiles of 128 cols)
            for ft in range(H):
                qt = qkvout.tile([128, N], bf16, tag=f"{r}qk{b}_{ft}")
                qk_sb[b][ft] = qt
                for tb in range(2):
                    ps = qkps.tile([128, 512], f32)
                    for c in range(CHUNKS):
                        nc.tensor.matmul(
                            ps[:], wqk_sb[c][:, ft * 128:(ft + 1) * 128],
                            xh[c][:, tb * 512:(tb + 1) * 512],
                            start=(c == 0), stop=(c == CHUNKS - 1))
                    nc.scalar.activation(qt[:, tb * 512:(tb + 1) * 512], ps[:],
                                         FP.Identity, bias=qkb_sb[ft][:])

            # -- V token-major with interleaved ones column
            for tt in range(TT):
                ps = vps.tile([128, DH], f32)
                for c in range(CHUNKS):
                    lhs = xh[c][:, tt * 128:(tt + 1) * 128]
                    nc.tensor.matmul(ps[:, 0:512], lhs, wv_sb[c][:, 0:512],
                                     start=(c == 0), stop=(c == CHUNKS - 1))
                    nc.tensor.matmul(ps[:, 512:DH], lhs, wv_sb[c][:, 512:DH],
                                     start=(c == 0), stop=(c == CHUNKS - 1))
                vt = qkvout.tile([128, H * 65], bf16, tag=f"{r}v{b}_{tt}")
                v_sb[b][tt] = vt
                vv = vt[:].rearrange("p (h d) -> p h d", d=65)
                nc.vector.tensor_add(
                    vv[:, :, 0:64],
                    ps[:].rearrange("p (h d) -> p h d", d=64),
                    vbB[:].rearrange("p (h d) -> p h d", d=65)[:, :, 0:64])
                nc.vector.tensor_copy(
                    vv[:, :, 64:65],
                    vbB[:].rearrange("p (h d) -> p h d", d=65)[:, :, 64:65])

    # ======== Phase 2: attention (p-outer, b-inner) ========
    with (
        tc.tile_pool(name="ebp", bufs=16) as ebp,
        tc.tile_pool(name="ep", bufs=18) as ep,
        tc.tile_pool(name="denrp", bufs=1) as denrp,
        tc.tile_pool(name="denc", bufs=2) as denc,
        tc.tile_pool(name="recp", bufs=2) as recp,
        tc.tile_pool(name="tmpb", bufs=2) as tmpb,
        tc.tile_pool(name="sps", bufs=2, space="PSUM") as spsp,
        tc.tile_pool(name="avps", bufs=4, space="PSUM") as avpsp,
    ):
        for p in range(PAIRS):
            eb_sb = [[None] * MT for _ in range(2)]
            for hh in range(2):
                for mt in range(MT):
                    t = ebp.tile([128, N], bf16)
                    nc.sync.dma_start(t[:], eb_d[2 * p + hh, mt])
                    eb_sb[hh][mt] = t
            for b in range(BL):
                QP = qk_sb[b][2 * p]
                KP = qk_sb[b][2 * p + 1]
                # scores + exp + bias-multiply
                E = [[None] * MT for _ in range(2)]
                for mt in range(MT):
                    for hh in range(2):
                        sl = slice(hh * 64, (hh + 1) * 64)
                        sps = spsp.tile([128, N], f32)
                        lhsT = KP[sl, mt * 128:(mt + 1) * 128]
                        nc.tensor.matmul(sps[:, 0:512], lhsT, QP[sl, 0:512],
                                         start=True, stop=True)
                        nc.tensor.matmul(sps[:, 512:N], lhsT, QP[sl, 512:N],
                                         start=True, stop=True)
                        et = ep.tile([128, N], bf16)
                        nc.scalar.activation(et[:], sps[:], FP.Exp, scale=SCALE)
                        nc.vector.tensor_mul(et[:], et[:], eb_sb[hh][mt][:])
                        E[hh][mt] = et
                # AV with ones column -> denominators in row 64
                denrow = denrp.tile([65, 4 * 512], f32)
                avt = [[None] * 2 for _ in range(2)]
                for hh in range(2):
                    h = 2 * p + hh
                    for nb in range(2):
                        avp = avpsp.tile([65, 512], f32)
                        for mt in range(MT):
                            vt = v_sb[b][mt]
                            lhsT = vt[:, h * 65:(h + 1) * 65]
                            nc.tensor.matmul(avp[:],
                                             lhsT,
                                             E[hh][mt][:, nb * 512:(nb + 1) * 512],
                                             start=(mt == 0), stop=(mt == MT - 1))
                        q = hh * 2 + nb
                        nc.scalar.copy(denrow[64:65, q * 512:(q + 1) * 512],
                                       avp[64:65, :])
                        avt[hh][nb] = avp
                # denominator reciprocal via DRAM bounce + exp(-ln(d))
                idx = p * BL + b
                nc.sync.dma_start(den_scr[idx:idx + 1, :], denrow[64:65, :])
                dc = denc.tile([4, 512], f32)
                nc.sync.dma_start(dc[:], den_scr[idx].rearrange("(q n) -> q n", n=512))
                lc = denc.tile([4, 512], f32)
                nc.scalar.activation(lc[:], dc[:], FP.Ln)
                rc = denc.tile([4, 512], bf16)
                nc.scalar.activation(rc[:], lc[:], FP.Exp, scale=-1.0)
                nc.sync.dma_start(rec_scr[idx].rearrange("(q n) -> q n", n=512), rc[:])
                recB = recp.tile([128, N], bf16)
                for hh in range(2):
                    for nb in range(2):
                        q = hh * 2 + nb
                        src = rec_scr[idx:idx + 1, q * 512:(q + 1) * 512]
                        nc.gpsimd.dma_start(
                            recB[hh * 64:(hh + 1) * 64, nb * 512:(nb + 1) * 512],
                            _bcast_ap(src, 64))
                # normalize into proj-input layout [(d_h0|d_h1), n]
                # reuse the (now dead) QPAIR slot of this (b, p) for proj input
                PI = qkvout.tile([128, N], bf16, tag=f"{r}qk{b}_{2 * p}")
                projIn[b][p] = PI
                tb_t = tmpb.tile([64, N], bf16)
                for nb in range(2):
                    ns = slice(nb * 512, (nb + 1) * 512)
                    nc.vector.tensor_mul(PI[0:64, ns], avt[0][nb][0:64, :],
                                         recB[0:64, ns])
                    nc.vector.tensor_mul(tb_t[:, ns], avt[1][nb][0:64, :],
                                         recB[64:128, ns])
                nc.sync.dma_start(PI[64:128, :], tb_t[:])

    # ======== Phase 3: output projection ========
    with (
        tc.tile_pool(name="outp", bufs=3) as outp,
        tc.tile_pool(name="pps", bufs=2, space="PSUM") as ppsp,
    ):
        for b in range(BL):
            for tt in range(TT):
                ps = ppsp.tile([128, DIM], f32)
                for p in range(PAIRS):
                    lhsT = projIn[b][p][:, tt * 128:(tt + 1) * 128]
                    nc.tensor.matmul(ps[:, 0:512], lhsT, pw_sb[p][:, 0:512],
                                     start=(p == 0), stop=(p == PAIRS - 1))
                    nc.tensor.matmul(ps[:, 512:DIM], lhsT, pw_sb[p][:, 512:DIM],
                                     start=(p == 0), stop=(p == PAIRS - 1))
                ot = outp.tile([128, DIM], f32)
                nc.vector.tensor_add(ot[:], ps[:], projbB[:])
                nc.sync.dma_start(out_d[b, tt * 128:(tt + 1) * 128, :], ot[:])


# ---------------- host side ----------------

def _prep_inputs(x, ln_w, ln_b, qkv_w, qkv_b, proj_w, proj_b,
                 attn_biases, bias_idxs):
    """Fold LN affine into QKV weights; build device layouts (shared part)."""
    f64 = np.float64
    Wp = qkv_w.astype(f64) * ln_w.astype(f64)[None, :]       # [2304, 768]
    bp = qkv_b.astype(f64) + qkv_w.astype(f64) @ ln_b.astype(f64)

    def q_rows(h): return np.arange(h * 3 * KD, h * 3 * KD + KD)
    def k_rows(h): return np.arange(h * 3 * KD + KD, h * 3 * KD + 2 * KD)
    def v_rows(h): return np.arange(h * 3 * KD + 2 * KD, h * 3 * KD + 3 * KD)

    qk_order = []
    for p in range(PAIRS):
        qk_order += list(q_rows(2 * p)) + list(q_rows(2 * p + 1))
        qk_order += list(k_rows(2 * p)) + list(k_rows(2 * p + 1))
    qk_order = np.array(qk_order)
    v_order = np.concatenate([v_rows(h) for h in range(H)])

    wqk = np.ascontiguousarray(Wp[qk_order].T).astype(ml_dtypes.bfloat16)
    qkb = bp[qk_order].astype(np.float32).reshape(H, 128, 1)
    wv = np.ascontiguousarray(Wp[v_order].T).astype(ml_dtypes.bfloat16)
    vb = np.zeros((1, H * 65), np.float32)
    vbv = vb.reshape(H, 65)
    vbv[:, 0:64] = bp[v_order].astype(np.float32).reshape(H, 64)
    vbv[:, 64] = 1.0  # ones column template
    pw = np.ascontiguousarray(proj_w.T).astype(ml_dtypes.bfloat16)
    pb = proj_b.astype(np.float32).reshape(1, DIM)

    ebias = np.exp(attn_biases.astype(f64))[:, np.asarray(bias_idxs)]  # [H,n,m]
    eb = np.ascontiguousarray(ebias.transpose(0, 2, 1)).astype(
        ml_dtypes.bfloat16).reshape(H, MT, 128, N)
    return dict(wqk=wqk, qkb=qkb, wv=wv, vb=vb, pw=pw, pb=pb, eb=eb)


def _make_in_maps(x, shared):
    xt = np.ascontiguousarray(
        x.reshape(NCORES, BL, N, DIM).transpose(0, 1, 3, 2)).astype(
            ml_dtypes.bfloat16)
    return [dict(x=xt[i], **shared) for i in range(NCORES)]


_PROG = {}


def _get_program(reps=1):
    if reps not in _PROG:
        _PROG[reps] = build_program(reps)
    return _PROG[reps]


def kernel(x, ln_w, ln_b, qkv_w, qkv_b, proj_w, proj_b,
           attn_biases, bias_idxs):
    from concourse.bass_utils import run_bass_kernel_spmd
    nc = _get_program()
    shared = _prep_inputs(x, ln_w, ln_b, qkv_w, qkv_b, proj_w, proj_b,
                          attn_biases, bias_idxs)
    in_maps = _make_in_maps(np.asarray(x), shared)
    res = run_bass_kernel_spmd(nc, in_maps, core_ids=list(range(NCORES)))
    out = np.concatenate([res.results[i]["out"] for i in range(NCORES)], axis=0)
    return out.astype(np.float32)


# revision 18
# speedup vs baseline: 10724.5502x; 10724.5502x over previous
"""Trainium2 Bass kernel for nn_Attention_62672162783289.

Dense transformer attention block: LayerNorm -> fused QKV -> per-head scaled
dot-product attention with gathered relative-position bias -> output proj.

Sharding: data-parallel over batch B=16 across 8 NeuronCores (2 batches/core).
No collectives needed; outputs are concatenated on the host.

Device-side design (per core, all matmuls bf16 with fp32 PSUM accumulation):
  - x is host-transposed to feature-major [2, 768, 1024] so LayerNorm stats are
    column sums computed with ones-matmuls; ln_w/ln_b are folded into the QKV
    weights on the host, so the device only applies (x - mu) * rstd.
  - rstd = exp(-0.5*ln(var+eps)) on ScalarE (DVE reciprocal is 8 cyc/elem).
  - Q,K are produced feature-major in head-pair tiles (q_2p|q_2p+1 /
    k_2p|k_2p+1 on partition halves) so score matmuls for the two heads of a
    pair run concurrently in different PE row groups (K=64 each).
  - Scores are computed transposed, S^T[m,n], so softmax normalization is a
    column sum obtained for free from an extra ones-column in the AV lhsT.
  - Softmax uses no max subtraction (scores are in [-2, 2] for this problem);
    the additive position bias is applied multiplicatively: exp(s+b) =
    exp(s)*exp(b) with exp(b) precomputed on the host (it is input-independent).
  - V is produced token-major [m, (h,65)] with an interleaved ones column so it
    can be used directly as the AV stationary operand.
  - AV output arrives transposed [(d|ones), n]; after dividing by the ones-row
    (denominator, bcast via DRAM bounce + exp(-ln(d))), it is exactly the
    proj contraction layout.
"""
import os
import numpy as np
import ml_dtypes

import concourse.bass as bass
import concourse.tile as tile
from concourse import bacc, mybir

bf16 = mybir.dt.bfloat16
f32 = mybir.dt.float32
FP = mybir.ActivationFunctionType

B, RES, DIM, H, KD = 16, 32, 768, 12, 64
N = RES * RES            # 1024 tokens
DH = KD * H              # 768
NCORES = 8
BL = B // NCORES         # 2 batches per core
SCALE = KD ** -0.5
LN_EPS = 1e-5
CHUNKS = DIM // 128      # 6 contraction chunks
PAIRS = H // 2           # 6 head pairs
MT = N // 128            # 8 m-tiles
TT = N // 128            # 8 token tiles


def _bcast_ap(dram_ap, nparts):
    """Partition-step-0 broadcast AP over a DRAM row region."""
    return bass.AP(tensor=dram_ap.tensor, offset=dram_ap.offset,
                   ap=[[0, nparts]] + dram_ap.ap[1:])


def build_program(reps=1):
    nc = bacc.Bacc("TRN2", target_bir_lowering=False, debug=False,
                   num_devices=NCORES)

    x_d = nc.dram_tensor("x", [BL, DIM, N], bf16, kind="ExternalInput").ap()
    wqk_d = nc.dram_tensor("wqk", [DIM, 2 * DH], bf16, kind="ExternalInput").ap()
    qkb_d = nc.dram_tensor("qkb", [H, 128, 1], f32, kind="ExternalInput").ap()
    wv_d = nc.dram_tensor("wv", [DIM, DH], bf16, kind="ExternalInput").ap()
    vb_d = nc.dram_tensor("vb", [1, H * 65], f32, kind="ExternalInput").ap()
    pw_d = nc.dram_tensor("pw", [DH, DIM], bf16, kind="ExternalInput").ap()
    pb_d = nc.dram_tensor("pb", [1, DIM], f32, kind="ExternalInput").ap()
    eb_d = nc.dram_tensor("eb", [H, MT, 128, N], bf16, kind="ExternalInput").ap()
    out_d = nc.dram_tensor("out", [BL, N, DIM], f32, kind="ExternalOutput").ap()

    den_scr = nc.dram_tensor("den_scr", [PAIRS * BL, 4 * 512], f32).ap()
    rec_scr = nc.dram_tensor("rec_scr", [PAIRS * BL, 4 * 512], bf16).ap()
    ab_scr = nc.dram_tensor("ab_scr", [BL, N], bf16).ap()
    bb_scr = nc.dram_tensor("bb_scr", [BL, N], bf16).ap()

    with tile.TileContext(nc) as tc:
        with (
            tc.tile_pool(name="persist", bufs=1) as persist,
            tc.tile_pool(name="qkvout", bufs=1) as qkvout,
        ):
            # ---- persistent weights / constants
            pw_sb = []
            for p in range(PAIRS):
                t = persist.tile([128, DIM], bf16, tag=f"pw{p}")
                nc.sync.dma_start(t[:], pw_d[p * 128:(p + 1) * 128, :])
                pw_sb.append(t)
            projbB = persist.tile([128, DIM], f32, tag="projbB")
            nc.gpsimd.dma_start(projbB[:], _bcast_ap(pb_d[0:1, :], 128))

            for rep in range(reps):
                _emit_body(nc, tc, persist, qkvout, rep,
                           x_d, wqk_d, qkb_d, wv_d, vb_d, eb_d, out_d,
                           den_scr, rec_scr, ab_scr, bb_scr, pw_sb, projbB)

    nc.compile()
    return nc


def _emit_body(nc, tc, persist, qkvout, rep,
               x_d, wqk_d, qkb_d, wv_d, vb_d, eb_d, out_d,
               den_scr, rec_scr, ab_scr, bb_scr, pw_sb, projbB):
    # tags are shared across reps: WAR deps serialize reps (correct; used for timing)
    r = ""
    qk_sb = [[None] * H for _ in range(BL)]  # 12 f-tiles: even=QPAIR, odd=KPAIR   # [b][ft] -> [128, N] bf16
    v_sb = [[None] * TT for _ in range(BL)]   # [b][tt] -> [128, H*65] bf16
    projIn = [[None] * PAIRS for _ in range(BL)]

    # ======== Phase 1: LayerNorm + QKV (both batches) ========
    with (
        tc.tile_pool(name="wqkp", bufs=1) as wqkp,
        tc.tile_pool(name="xp", bufs=7) as xp,
        tc.tile_pool(name="sqp", bufs=2) as sqp,
        tc.tile_pool(name="rowp", bufs=1) as rowp,
        tc.tile_pool(name="lnbc", bufs=2) as lnbc,
        tc.tile_pool(name="xhp", bufs=12) as xhp,
        tc.tile_pool(name="tmpp", bufs=2) as tmpp,
        tc.tile_pool(name="lnps", bufs=2, space="PSUM") as lnps,
        tc.tile_pool(name="qkps", bufs=2, space="PSUM") as qkps,
        tc.tile_pool(name="vps", bufs=2, space="PSUM") as vps,
    ):
        wqk_sb, wv_sb, qkb_sb = [], [], []
        for c in range(CHUNKS):
            t = wqkp.tile([128, 2 * DH], bf16, tag=f"wqk{c}")
            nc.sync.dma_start(t[:], wqk_d[c * 128:(c + 1) * 128, :])
            wqk_sb.append(t)
            t = wqkp.tile([128, DH], bf16, tag=f"wv{c}")
            nc.sync.dma_start(t[:], wv_d[c * 128:(c + 1) * 128, :])
            wv_sb.append(t)
        for ft in range(H):
            t = wqkp.tile([128, 1], f32, tag=f"qkb{ft}")
            nc.sync.dma_start(t[:], qkb_d[ft])
            qkb_sb.append(t)
        vbB = wqkp.tile([128, H * 65], f32, tag="vbB")
        nc.gpsimd.dma_start(vbB[:], _bcast_ap(vb_d[0:1, :], 128))
        ones_c = wqkp.tile([128, 1], bf16, tag="ones")
        nc.vector.memset(ones_c[:], 1.0)
        eps_t = wqkp.tile([128, 1], f32, tag="eps")
        nc.vector.memset(eps_t[:], LN_EPS)

        for b in range(BL):
            # -- load x chunks (feature-major)
            xc = []
            for c in range(CHUNKS):
                t = xp.tile([128, N], bf16)
                nc.gpsimd.dma_start(t[:], x_d[b, c * 128:(c + 1) * 128, :])
                xc.append(t)

            # -- stats: column sums of x and x^2 via ones-matmuls
            mrow = rowp.tile([1, N], f32, tag="mrow")
            srow = rowp.tile([1, N], f32, tag="srow")
            for tb in range(2):
                mean_ps = lnps.tile([1, 512], f32, tag="lnstat")
                sq_ps = lnps.tile([1, 512], f32, tag="lnstat")
                for c in range(CHUNKS):
                    sl = xc[c][:, tb * 512:(tb + 1) * 512]
                    sq = sqp.tile([128, 512], bf16)
                    nc.scalar.activation(sq[:], sl, FP.Square)
                    nc.tensor.matmul(mean_ps[:], ones_c[:], sl,
                                     start=(c == 0), stop=(c == CHUNKS - 1))
                    nc.tensor.matmul(sq_ps[:], ones_c[:], sq[:],
                                     start=(c == 0), stop=(c == CHUNKS - 1))
                nc.scalar.copy(mrow[0:1, tb * 512:(tb + 1) * 512], mean_ps[:])
                nc.scalar.copy(srow[0:1, tb * 512:(tb + 1) * 512], sq_ps[:])

            # -- row math: alpha = rstd, beta = -mu*rstd (1 lane, in-place)
            nc.vector.tensor_scalar(mrow[:], mrow[:], 1.0 / DIM, None,
                                    mybir.AluOpType.mult)  # mrow = mu
            mu2 = rowp.tile([1, N], f32, tag="mu2")
            nc.vector.tensor_mul(mu2[:], mrow[:], mrow[:])
            nc.vector.scalar_tensor_tensor(srow[:], srow[:], 1.0 / DIM, mu2[:],
                                           mybir.AluOpType.mult,
                                           mybir.AluOpType.subtract)  # srow = var
            nc.scalar.activation(srow[:], srow[:], FP.Ln, bias=eps_t[0:1, :])
            arow = rowp.tile([1, N], bf16, tag="arow")
            nc.scalar.activation(arow[:], srow[:], FP.Exp, scale=-0.5)
            brow = rowp.tile([1, N], bf16, tag="brow")
            nc.vector.scalar_tensor_tensor(brow[:], mrow[:], -1.0, arow[:],
                                           mybir.AluOpType.mult,
                                           mybir.AluOpType.mult)
            nc.sync.dma_start(ab_scr[b:b + 1, :], arow[:])
            nc.sync.dma_start(bb_scr[b:b + 1, :], brow[:])
            alphaB = lnbc.tile([128, N], bf16, tag="alphaB")
            nc.gpsimd.dma_start(alphaB[:], _bcast_ap(ab_scr[b:b + 1, :], 128))
            betaB = lnbc.tile([128, N], bf16, tag="betaB")
            nc.gpsimd.dma_start(betaB[:], _bcast_ap(bb_scr[b:b + 1, :], 128))

            # -- xhat = x*alpha + beta (bf16)
            xh = []
            for c in range(CHUNKS):
                t0 = tmpp.tile([128, N], bf16)
                nc.vector.tensor_mul(t0[:], xc[c][:], alphaB[:])
                t = xhp.tile([128, N], bf16)
                nc.vector.tensor_add(t[:], t0[:], betaB[:])
                xh.append(t)

            # -- Q,K feature-major head-pair tiles (12 f-tiles of 128 cols)
            for ft in range(H):
                qt = qkvout.tile([128, N], bf16, tag=f"{r}qk{b}_{ft}")
                qk_sb[b][ft] = qt
                for tb in range(2):
                    ps = qkps.tile([128, 512], f32)
                    for c in range(CHUNKS):
                        nc.tensor.matmul(
                            ps[:], wqk_sb[c][:, ft * 128:(ft + 1) * 128],
                            xh[c][:, tb * 512:(tb + 1) * 512],
                            start=(c == 0), stop=(c == CHUNKS - 1))
                    nc.scalar.activation(qt[:, tb * 512:(tb + 1) * 512], ps[:],
                                         FP.Identity, bias=qkb_sb[ft][:])

            # -- V token-major with interleaved ones column
            for tt in range(TT):
                ps = vps.tile([128, DH], f32)
                for c in range(CHUNKS):
                    lhs = xh[c][:, tt * 128:(tt + 1) * 128]
                    nc.tensor.matmul(ps[:, 0:512], lhs, wv_sb[c][:, 0:512],
                                     start=(c == 0), stop=(c == CHUNKS - 1))
                    nc.tensor.matmul(ps[:, 512:DH], lhs, wv_sb[c][:, 512:DH],
                                     start=(c == 0), stop=(c == CHUNKS - 1))
                vt = qkvout.tile([128, H * 65], bf16, tag=f"{r}v{b}_{tt}")
                v_sb[b][tt] = vt
                vv = vt[:].rearrange("p (h d) -> p h d", d=65)
                nc.vector.tensor_add(
                    vv[:, :, 0:64],
                    ps[:].rearrange("p (h d) -> p h d", d=64),
                    vbB[:].rearrange("p (h d) -> p h d", d=65)[:, :, 0:64])
                nc.vector.tensor_copy(
                    vv[:, :, 64:65],
                    vbB[:].rearrange("p (h d) -> p h d", d=65)[:, :, 64:65])

    # ======== Phase 2: attention (p-outer, b-inner) ========
    # Software-pipelined across (p, b) steps:
    #   step i emits: scores+exp+bias-mul of i, AV matmuls of i-1 (interleaved
    #   into the PE stream per m-chunk), the deferred reciprocal+normalize of
    #   i-2 (mid-step, so the DVE never stalls on the broadcast DMAs), and the
    #   PSUM evacuation of i-1 (unnormalized, so the AV accumulator banks free
    #   quickly).
    with (
        tc.tile_pool(name="ebp", bufs=16) as ebp,
        tc.tile_pool(name="ep", bufs=20) as ep,
        tc.tile_pool(name="denrp", bufs=2) as denrp,
        tc.tile_pool(name="recp", bufs=3) as recp,
        tc.tile_pool(name="tmpb", bufs=2) as tmpb,
        tc.tile_pool(name="sps", bufs=2, space="PSUM") as spsp,
        tc.tile_pool(name="avps", bufs=4, space="PSUM") as avpsp,
    ):
        steps = [(p, b) for p in range(PAIRS) for b in range(BL)]
        eb_cache = {}

        def load_eb(p):
            eb_sb = [[None] * MT for _ in range(2)]
            for hh in range(2):
                for mt in range(MT):
                    t = ebp.tile([128, N], bf16)
                    nc.gpsimd.dma_start(t[:], eb_d[2 * p + hh, mt])
                    eb_sb[hh][mt] = t
            return eb_sb

        def av_chunk(st, mt):
            """Emit the 4 AV matmuls of pipeline-state `st` for m-chunk mt."""
            p, b, E, avt = st["p"], st["b"], st["E"], st["avt"]
            for hh in range(2):
                h = 2 * p + hh
                lhsT = v_sb[b][mt][:, h * 65:(h + 1) * 65]
                for nb in range(2):
                    nc.tensor.matmul(avt[hh][nb][:], lhsT,
                                     E[hh][mt][:, nb * 512:(nb + 1) * 512],
                                     start=(mt == 0), stop=(mt == MT - 1))

        def evac_step(st):
            """Denominator rows out + unnormalized PSUM->SBUF evacuation
            (frees the AV accumulator banks) + reciprocal-broadcast DMAs."""
            p, b, avt = st["p"], st["b"], st["avt"]
            denrow = denrp.tile([65, 4 * 512], f32)
            for hh in range(2):
                for nb in range(2):
                    q = hh * 2 + nb
                    eng = nc.scalar if hh == 0 else nc.vector
                    if hh == 0:
                        nc.scalar.copy(denrow[64:65, q * 512:(q + 1) * 512],
                                       avt[hh][nb][64:65, :])
                    else:
                        nc.vector.tensor_copy(
                            denrow[64:65, q * 512:(q + 1) * 512],
                            avt[hh][nb][64:65, :])
            # unnormalized attention output, proj-input layout
            PI = qkvout.tile([128, N], bf16, tag=f"{r}qk{b}_{2 * p}")
            projIn[b][p] = PI
            tb_t = tmpb.tile([64, N], bf16)
            for nb in range(2):
                ns = slice(nb * 512, (nb + 1) * 512)
                nc.vector.tensor_copy(PI[0:64, ns], avt[0][nb][0:64, :])
                nc.scalar.copy(tb_t[:, ns], avt[1][nb][0:64, :])
            nc.sync.dma_start(PI[64:128, :], tb_t[:])
            # denominator bounce + broadcast (completes off the critical path)
            idx = p * BL + b
            nc.sync.dma_start(den_scr[idx:idx + 1, :], denrow[64:65, :])
            denB = recp.tile([128, N], f32, tag="recp")
            for hh in range(2):
                for nb in range(2):
                    q = hh * 2 + nb
                    src = den_scr[idx:idx + 1, q * 512:(q + 1) * 512]
                    nc.gpsimd.dma_start(
                        denB[hh * 64:(hh + 1) * 64, nb * 512:(nb + 1) * 512],
                        _bcast_ap(src, 64))
            st["denB"] = denB
            st["PI"] = PI

        def norm_step(st):
            """In-place normalize the unnormalized proj input of `st`."""
            rB = recp.tile([128, N], f32, tag="recp")
            nc.vector.reciprocal_approx_fast(rB[:], st["denB"][:])
            nc.vector.tensor_mul(st["PI"][:], st["PI"][:], rB[:])

        prev = None
        pend = None
        for p, b in steps:
            if b == 0:
                eb_cache[p] = load_eb(p)
            eb_sb = eb_cache[p]
            QP = qk_sb[b][2 * p]
            KP = qk_sb[b][2 * p + 1]
            E = [[None] * MT for _ in range(2)]
            cur = {"p": p, "b": b, "E": E,
                   "avt": [[avpsp.tile([65, 512], f32, name="avt", tag="avt")
                            for _ in range(2)] for _ in range(2)]}
            for mt in range(MT):
                for hh in range(2):
                    sl = slice(hh * 64, (hh + 1) * 64)
                    sps = spsp.tile([128, N], f32)
                    lhsT = KP[sl, mt * 128:(mt + 1) * 128]
                    nc.tensor.matmul(sps[:, 0:512], lhsT, QP[sl, 0:512],
                                     start=True, stop=True)
                    nc.tensor.matmul(sps[:, 512:N], lhsT, QP[sl, 512:N],
                                     start=True, stop=True)
                    et = ep.tile([128, N], bf16)
                    nc.scalar.activation(et[:], sps[:], FP.Exp, scale=SCALE)
                    nc.vector.tensor_mul(et[:], et[:], eb_sb[hh][mt][:])
                    E[hh][mt] = et
                if prev is not None:
                    av_chunk(prev, mt)
                if mt == 3 and pend is not None:
                    norm_step(pend)
                    pend = None
            if prev is not None:
                evac_step(prev)
                pend = prev
            prev = cur
        # drain the pipeline
        for mt in range(MT):
            av_chunk(prev, mt)
            if mt == 3 and pend is not None:
                norm_step(pend)
                pend = None
        evac_step(prev)
        norm_step(prev)

    # ======== Phase 3: output projection ========
    with (
        tc.tile_pool(name="outp", bufs=3) as outp,
        tc.tile_pool(name="pps", bufs=2, space="PSUM") as ppsp,
    ):
        for b in range(BL):
            for tt in range(TT):
                ps = ppsp.tile([128, DIM], f32)
                for p in range(PAIRS):
                    lhsT = projIn[b][p][:, tt * 128:(tt + 1) * 128]
                    nc.tensor.matmul(ps[:, 0:512], lhsT, pw_sb[p][:, 0:512],
                                     start=(p == 0), stop=(p == PAIRS - 1))
                    nc.tensor.matmul(ps[:, 512:DIM], lhsT, pw_sb[p][:, 512:DIM],
                                     start=(p == 0), stop=(p == PAIRS - 1))
                ot = outp.tile([128, DIM], f32)
                nc.vector.tensor_add(ot[:], ps[:], projbB[:])
                nc.scalar.dma_start(out_d[b, tt * 128:(tt + 1) * 128, :], ot[:])


# ---------------- host side ----------------

def _prep_inputs(x, ln_w, ln_b, qkv_w, qkv_b, proj_w, proj_b,
                 attn_biases, bias_idxs):
    """Fold LN affine into QKV weights; build device layouts (shared part)."""
    f64 = np.float64
    Wp = qkv_w.astype(f64) * ln_w.astype(f64)[None, :]       # [2304, 768]
    bp = qkv_b.astype(f64) + qkv_w.astype(f64) @ ln_b.astype(f64)

    def q_rows(h): return np.arange(h * 3 * KD, h * 3 * KD + KD)
    def k_rows(h): return np.arange(h * 3 * KD + KD, h * 3 * KD + 2 * KD)
    def v_rows(h): return np.arange(h * 3 * KD + 2 * KD, h * 3 * KD + 3 * KD)

    qk_order = []
    for p in range(PAIRS):
        qk_order += list(q_rows(2 * p)) + list(q_rows(2 * p + 1))
        qk_order += list(k_rows(2 * p)) + list(k_rows(2 * p + 1))
    qk_order = np.array(qk_order)
    v_order = np.concatenate([v_rows(h) for h in range(H)])

    wqk = np.ascontiguousarray(Wp[qk_order].T).astype(ml_dtypes.bfloat16)
    qkb = bp[qk_order].astype(np.float32).reshape(H, 128, 1)
    wv = np.ascontiguousarray(Wp[v_order].T).astype(ml_dtypes.bfloat16)
    vb = np.zeros((1, H * 65), np.float32)
    vbv = vb.reshape(H, 65)
    vbv[:, 0:64] = bp[v_order].astype(np.float32).reshape(H, 64)
    vbv[:, 64] = 1.0  # ones column template
    pw = np.ascontiguousarray(proj_w.T).astype(ml_dtypes.bfloat16)
    pb = proj_b.astype(np.float32).reshape(1, DIM)

    ebias = np.exp(attn_biases.astype(f64))[:, np.asarray(bias_idxs)]  # [H,n,m]
    eb = np.ascontiguousarray(ebias.transpose(0, 2, 1)).astype(
        ml_dtypes.bfloat16).reshape(H, MT, 128, N)
    return dict(wqk=wqk, qkb=qkb, wv=wv, vb=vb, pw=pw, pb=pb, eb=eb)


def _make_in_maps(x, shared):
    xt = np.ascontiguousarray(
        x.reshape(NCORES, BL, N, DIM).transpose(0, 1, 3, 2)).astype(
            ml_dtypes.bfloat16)
    return [dict(x=xt[i], **shared) for i in range(NCORES)]


_PROG = {}


def _get_program(reps=1):
    if reps not in _PROG:
        _PROG[reps] = build_program(reps)
    return _PROG[reps]


class _Runner:
    """Persistent jitted SPMD executor (mirrors bass2jax.run_bass_via_pjrt's
    multi-core branch, but the jitted callable is cached across calls)."""

    def __init__(self, nc):
        import jax
        from jax.experimental.shard_map import shard_map
        from jax.sharding import Mesh, PartitionSpec
        from concourse import mybir as _mb
        from concourse.bass2jax import _bass_exec_p, install_neuronx_cc_hook

        install_neuronx_cc_hook()
        self.jax = jax
        from concourse.bass2jax import partition_id_tensor
        part_name = (nc.partition_id_tensor.name
                     if nc.partition_id_tensor else None)
        in_names, out_names, out_avals = [], [], []
        for alloc in nc.m.functions[0].allocations:
            if not isinstance(alloc, _mb.MemoryLocationSet):
                continue
            name = alloc.memorylocations[0].name
            if alloc.kind == "ExternalInput":
                if name != part_name:
                    in_names.append(name)
            elif alloc.kind == "ExternalOutput":
                out_names.append(name)
                out_avals.append(jax.core.ShapedArray(
                    tuple(alloc.tensor_shape), _mb.dt.np(alloc.dtype)))
        self.in_names, self.out_names, self.out_avals = in_names, out_names, out_avals
        n_params, n_outs = len(in_names), len(out_names)
        bind_names = tuple(in_names + out_names
                           + ([part_name] if part_name else []))

        def _body(*args):
            operands = list(args)
            if part_name:
                operands.append(partition_id_tensor())
            return tuple(_bass_exec_p.bind(
                *operands, out_avals=tuple(out_avals), in_names=bind_names,
                out_names=tuple(out_names), lowering_input_output_aliases=(),
                sim_require_finite=True, sim_require_nnan=True, nc=nc))

        devices = jax.devices()[:NCORES]
        self.mesh = Mesh(np.asarray(devices), ("core",))
        in_specs = (PartitionSpec("core"),) * (n_params + n_outs)
        out_specs = (PartitionSpec("core"),) * n_outs
        self.sharded = jax.jit(
            shard_map(_body, mesh=self.mesh, in_specs=in_specs,
                      out_specs=out_specs, check_rep=False),
            donate_argnums=tuple(range(n_params, n_params + n_outs)),
            keep_unused=True)
        self.sharding = jax.sharding.NamedSharding(
            self.mesh, PartitionSpec("core"))

    def put_inputs(self, in_maps):
        """Concatenate per-core inputs on axis 0 and place on devices."""
        concat = [np.concatenate([np.asarray(m[n]) for m in in_maps], axis=0)
                  for n in self.in_names]
        return [self.jax.device_put(a, self.sharding) for a in concat]

    def zeros(self):
        return [self.jax.device_put(
                    np.zeros((NCORES * av.shape[0], *av.shape[1:]), av.dtype),
                    self.sharding)
                for av in self.out_avals]

    def run(self, dev_inputs, dev_zeros=None):
        if dev_zeros is None:
            dev_zeros = self.zeros()
        outs = self.sharded(*dev_inputs, *dev_zeros)
        self.jax.block_until_ready(outs)
        return outs

    def run_np(self, dev_inputs):
        outs = self.run(dev_inputs)
        res = {}
        for i, name in enumerate(self.out_names):
            a = np.asarray(outs[i])
            res[name] = a.reshape(NCORES, *self.out_avals[i].shape)
        return res


_RUNNERS = {}


def _get_runner(reps=1):
    if reps not in _RUNNERS:
        _RUNNERS[reps] = _Runner(_get_program(reps))
    return _RUNNERS[reps]


def kernel(x, ln_w, ln_b, qkv_w, qkv_b, proj_w, proj_b,
           attn_biases, bias_idxs):
    runner = _get_runner()
    shared = _prep_inputs(x, ln_w, ln_b, qkv_w, qkv_b, proj_w, proj_b,
                          attn_biases, bias_idxs)
    in_maps = _make_in_maps(np.asarray(x), shared)
    dev = runner.put_inputs(in_maps)
    out = runner.run_np(dev)["out"]          # [NCORES, BL, N, DIM]
    return out.reshape(B, N, DIM).astype(np.float32)


# revision 24
# speedup vs baseline: 10961.3598x; 1.0221x over previous
"""Trainium2 Bass kernel for nn_Attention_62672162783289.

Dense transformer attention block: LayerNorm -> fused QKV -> per-head scaled
dot-product attention with gathered relative-position bias -> output proj.

Sharding: data-parallel over batch B=16 across 8 NeuronCores (2 batches/core).
No collectives needed; outputs are concatenated on the host.

Device-side design (per core, all matmuls bf16 with fp32 PSUM accumulation):
  - x is host-transposed to feature-major [2, 768, 1024] so LayerNorm stats are
    column sums computed with ones-matmuls; ln_w/ln_b are folded into the QKV
    weights on the host, so the device only applies (x - mu) * rstd.
  - rstd = exp(-0.5*ln(var+eps)) on ScalarE (DVE reciprocal is 8 cyc/elem).
  - Q,K are produced feature-major in head-pair tiles (q_2p|q_2p+1 /
    k_2p|k_2p+1 on partition halves) so score matmuls for the two heads of a
    pair run concurrently in different PE row groups (K=64 each).
  - Scores are computed transposed, S^T[m,n], so softmax normalization is a
    column sum obtained for free from an extra ones-column in the AV lhsT.
  - Softmax uses no max subtraction (scores are in [-2, 2] for this problem);
    the additive position bias is applied multiplicatively: exp(s+b) =
    exp(s)*exp(b) with exp(b) precomputed on the host (it is input-independent).
  - V is produced token-major [m, (h,65)] with an interleaved ones column so it
    can be used directly as the AV stationary operand.
  - AV output arrives transposed [(d|ones), n]; after dividing by the ones-row
    (denominator, bcast via DRAM bounce + exp(-ln(d))), it is exactly the
    proj contraction layout.
"""
import os
import numpy as np
import ml_dtypes

import concourse.bass as bass
import concourse.tile as tile
from concourse import bacc, mybir

bf16 = mybir.dt.bfloat16
f32 = mybir.dt.float32
FP = mybir.ActivationFunctionType

B, RES, DIM, H, KD = 16, 32, 768, 12, 64
N = RES * RES            # 1024 tokens
DH = KD * H              # 768
NCORES = 8
BL = B // NCORES         # 2 batches per core
SCALE = KD ** -0.5
LN_EPS = 1e-5
CHUNKS = DIM // 128      # 6 contraction chunks
PAIRS = H // 2           # 6 head pairs
MT = N // 128            # 8 m-tiles
TT = N // 128            # 8 token tiles


def _bcast_ap(dram_ap, nparts):
    """Partition-step-0 broadcast AP over a DRAM row region."""
    return bass.AP(tensor=dram_ap.tensor, offset=dram_ap.offset,
                   ap=[[0, nparts]] + dram_ap.ap[1:])


def build_program(reps=1):
    nc = bacc.Bacc("TRN2", target_bir_lowering=False, debug=False,
                   num_devices=NCORES)

    x_d = nc.dram_tensor("x", [BL, DIM, N], bf16, kind="ExternalInput").ap()
    wqk_d = nc.dram_tensor("wqk", [DIM, 2 * DH], bf16, kind="ExternalInput").ap()
    qkb_d = nc.dram_tensor("qkb", [H, 128, 1], f32, kind="ExternalInput").ap()
    wv_d = nc.dram_tensor("wv", [DIM, DH], bf16, kind="ExternalInput").ap()
    vb_d = nc.dram_tensor("vb", [1, H * 65], f32, kind="ExternalInput").ap()
    pw_d = nc.dram_tensor("pw", [DH, DIM], bf16, kind="ExternalInput").ap()
    pb_d = nc.dram_tensor("pb", [1, DIM], f32, kind="ExternalInput").ap()
    eb_d = nc.dram_tensor("eb", [H, MT, 128, N], bf16, kind="ExternalInput").ap()
    out_d = nc.dram_tensor("out", [BL, N, DIM], f32, kind="ExternalOutput").ap()

    den_scr = nc.dram_tensor("den_scr", [PAIRS * BL, 4 * 512], f32).ap()
    rec_scr = nc.dram_tensor("rec_scr", [PAIRS * BL, 4 * 512], bf16).ap()
    ab_scr = nc.dram_tensor("ab_scr", [BL, N], bf16).ap()
    bb_scr = nc.dram_tensor("bb_scr", [BL, N], bf16).ap()

    with tile.TileContext(nc) as tc:
        with (
            tc.tile_pool(name="persist", bufs=1) as persist,
            tc.tile_pool(name="qkvout", bufs=1) as qkvout,
        ):
            # ---- persistent weights / constants
            pw_sb = []
            for p in range(PAIRS):
                t = persist.tile([128, DIM], bf16, tag=f"pw{p}")
                nc.sync.dma_start(t[:], pw_d[p * 128:(p + 1) * 128, :])
                pw_sb.append(t)
            projbB = persist.tile([128, DIM], f32, tag="projbB")
            nc.gpsimd.dma_start(projbB[:], _bcast_ap(pb_d[0:1, :], 128))

            for rep in range(reps):
                _emit_body(nc, tc, persist, qkvout, rep,
                           x_d, wqk_d, qkb_d, wv_d, vb_d, eb_d, out_d,
                           den_scr, rec_scr, ab_scr, bb_scr, pw_sb, projbB)

    nc.compile()
    return nc


def _emit_body(nc, tc, persist, qkvout, rep,
               x_d, wqk_d, qkb_d, wv_d, vb_d, eb_d, out_d,
               den_scr, rec_scr, ab_scr, bb_scr, pw_sb, projbB):
    # tags are shared across reps: WAR deps serialize reps (correct; for timing)
    r = ""
    qk_sb = [[None] * H for _ in range(BL)]  # 12 f-tiles: even=QPAIR, odd=KPAIR
    v_sb = [[None] * TT for _ in range(BL)]  # [b][tt] -> [128, H*65] bf16
    projIn = [[None] * PAIRS for _ in range(BL)]

    # ======== Phase 1: LayerNorm + QKV, staged so both batches' LN chains
    # overlap QKV(b0): A=stats, B=row math+bcast, C=xhat, D=QKV ========
    with (
        tc.tile_pool(name="wqkp", bufs=1) as wqkp,
        tc.tile_pool(name="xp", bufs=12) as xp,
        tc.tile_pool(name="sqp", bufs=2) as sqp,
        tc.tile_pool(name="rowp", bufs=1) as rowp,
        tc.tile_pool(name="lnbc", bufs=2) as lnbc,
        tc.tile_pool(name="xhp", bufs=12) as xhp,
        tc.tile_pool(name="tmpp", bufs=2) as tmpp,
        tc.tile_pool(name="lnps", bufs=2, space="PSUM") as lnps,
        tc.tile_pool(name="qkps", bufs=2, space="PSUM") as qkps,
        tc.tile_pool(name="vps", bufs=2, space="PSUM") as vps,
    ):
        # x first on the sync queue (needed within ~5us), weights after
        xc = [[None] * CHUNKS for _ in range(BL)]
        for b in range(BL):
            for c in range(CHUNKS):
                t = xp.tile([128, N], bf16, name="xc", tag="xc")
                nc.sync.dma_start(t[:], x_d[b, c * 128:(c + 1) * 128, :])
                xc[b][c] = t
        wqk_sb, wv_sb, qkb_sb = [], [], []
        for c in range(CHUNKS):
            t = wqkp.tile([128, 2 * DH], bf16, tag=f"wqk{c}")
            nc.sync.dma_start(t[:], wqk_d[c * 128:(c + 1) * 128, :])
            wqk_sb.append(t)
            t = wqkp.tile([128, DH], bf16, tag=f"wv{c}")
            nc.sync.dma_start(t[:], wv_d[c * 128:(c + 1) * 128, :])
            wv_sb.append(t)
        for ft in range(H):
            t = wqkp.tile([128, 1], f32, tag=f"qkb{ft}")
            nc.sync.dma_start(t[:], qkb_d[ft])
            qkb_sb.append(t)
        vbB = wqkp.tile([128, H * 65], f32, tag="vbB")
        nc.gpsimd.dma_start(vbB[:], _bcast_ap(vb_d[0:1, :], 128))
        ones_c = wqkp.tile([128, 1], bf16, tag="ones")
        nc.vector.memset(ones_c[:], 1.0)
        eps_t = wqkp.tile([128, 1], f32, tag="eps")
        nc.vector.memset(eps_t[:], LN_EPS)

        rows = {}

        def stage_a(b):  # stats: column sums of x and x^2 via ones-matmuls
            mrow = rowp.tile([1, N], f32, tag="mrow")
            srow = rowp.tile([1, N], f32, tag="srow")
            for tb in range(2):
                mean_ps = lnps.tile([1, 512], f32, tag="lnstat")
                sq_ps = lnps.tile([1, 512], f32, tag="lnstat")
                for c in range(CHUNKS):
                    sl = xc[b][c][:, tb * 512:(tb + 1) * 512]
                    sq = sqp.tile([128, 512], bf16)
                    nc.scalar.activation(sq[:], sl, FP.Square)
                    nc.tensor.matmul(mean_ps[:], ones_c[:], sl,
                                     start=(c == 0), stop=(c == CHUNKS - 1))
                    nc.tensor.matmul(sq_ps[:], ones_c[:], sq[:],
                                     start=(c == 0), stop=(c == CHUNKS - 1))
                nc.scalar.copy(mrow[0:1, tb * 512:(tb + 1) * 512], mean_ps[:])
                nc.scalar.copy(srow[0:1, tb * 512:(tb + 1) * 512], sq_ps[:])
            rows[b] = (mrow, srow)

        def stage_b(b):  # row math (1 lane, in-place) + DRAM-bounce broadcast
            mrow, srow = rows[b]
            nc.vector.tensor_scalar(mrow[:], mrow[:], 1.0 / DIM, None,
                                    mybir.AluOpType.mult)  # mrow = mu
            mu2 = rowp.tile([1, N], f32, tag="mu2")
            nc.vector.tensor_mul(mu2[:], mrow[:], mrow[:])
            nc.vector.scalar_tensor_tensor(srow[:], srow[:], 1.0 / DIM, mu2[:],
                                           mybir.AluOpType.mult,
                                           mybir.AluOpType.subtract)  # = var
            nc.scalar.activation(srow[:], srow[:], FP.Ln, bias=eps_t[0:1, :])
            arow = rowp.tile([1, N], bf16, tag="arow")
            nc.scalar.activation(arow[:], srow[:], FP.Exp, scale=-0.5)
            brow = rowp.tile([1, N], bf16, tag="brow")
            nc.vector.scalar_tensor_tensor(brow[:], mrow[:], -1.0, arow[:],
                                           mybir.AluOpType.mult,
                                           mybir.AluOpType.mult)
            nc.sync.dma_start(ab_scr[b:b + 1, :], arow[:])
            nc.sync.dma_start(bb_scr[b:b + 1, :], brow[:])
            alphaB = lnbc.tile([128, N], bf16, tag="alphaB")
            nc.gpsimd.dma_start(alphaB[:], _bcast_ap(ab_scr[b:b + 1, :], 128))
            betaB = lnbc.tile([128, N], bf16, tag="betaB")
            nc.gpsimd.dma_start(betaB[:], _bcast_ap(bb_scr[b:b + 1, :], 128))
            rows[b] = (alphaB, betaB)

        xh = [[None] * CHUNKS for _ in range(BL)]

        def stage_c(b):  # xhat = x*alpha + beta (bf16)
            alphaB, betaB = rows[b]
            for c in range(CHUNKS):
                t0 = tmpp.tile([128, N], bf16, name="t0", tag="t0")
                nc.vector.tensor_mul(t0[:], xc[b][c][:], alphaB[:])
                t = xhp.tile([128, N], bf16, name="xh", tag="xh")
                nc.vector.tensor_add(t[:], t0[:], betaB[:])
                xh[b][c] = t

        def stage_d(b):  # QKV matmuls
            for ft in range(H):
                qt = qkvout.tile([128, N], bf16, tag=f"{r}qk{b}_{ft}")
                qk_sb[b][ft] = qt
                for tb in range(2):
                    ps = qkps.tile([128, 512], f32, name="qkp", tag="qkp")
                    for c in range(CHUNKS):
                        nc.tensor.matmul(
                            ps[:], wqk_sb[c][:, ft * 128:(ft + 1) * 128],
                            xh[b][c][:, tb * 512:(tb + 1) * 512],
                            start=(c == 0), stop=(c == CHUNKS - 1))
                    nc.scalar.activation(qt[:, tb * 512:(tb + 1) * 512], ps[:],
                                         FP.Identity, bias=qkb_sb[ft][:])
            for tt in range(TT):
                ps = vps.tile([128, DH], f32, name="vp", tag="vp")
                for c in range(CHUNKS):
                    lhs = xh[b][c][:, tt * 128:(tt + 1) * 128]
                    nc.tensor.matmul(ps[:, 0:512], lhs, wv_sb[c][:, 0:512],
                                     start=(c == 0), stop=(c == CHUNKS - 1))
                    nc.tensor.matmul(ps[:, 512:DH], lhs, wv_sb[c][:, 512:DH],
                                     start=(c == 0), stop=(c == CHUNKS - 1))
                vt = qkvout.tile([128, H * 65], bf16, tag=f"{r}v{b}_{tt}")
                v_sb[b][tt] = vt
                vv = vt[:].rearrange("p (h d) -> p h d", d=65)
                nc.vector.tensor_add(
                    vv[:, :, 0:64],
                    ps[:].rearrange("p (h d) -> p h d", d=64),
                    vbB[:].rearrange("p (h d) -> p h d", d=65)[:, :, 0:64])
                nc.vector.tensor_copy(
                    vv[:, :, 64:65],
                    vbB[:].rearrange("p (h d) -> p h d", d=65)[:, :, 64:65])

        stage_a(0); stage_b(0); stage_a(1); stage_c(0); stage_b(1)
        stage_c(1); stage_d(0); stage_d(1)

    # ======== Phase 2: attention + projection, software-pipelined ========
    # Step i emits: scores+exp+bias-mul of i with the AV matmuls of i-1
    # interleaved per m-chunk (PE never idles long enough for HAM to
    # re-throttle), the deferred reciprocal+normalize of i-2 mid-step, and the
    # AV-accumulator evacuation of i-1. proj(b0) is emitted before the
    # pipeline drain so the tensor engine has dense work while the last
    # step's exps stream through ScalarE.
    with (
        tc.tile_pool(name="ebp", bufs=10) as ebp,
        tc.tile_pool(name="ep", bufs=10) as ep,
        tc.tile_pool(name="denrp", bufs=1) as denrp,
        tc.tile_pool(name="recp", bufs=3) as recp,
        tc.tile_pool(name="tmpb", bufs=2) as tmpb,
        tc.tile_pool(name="outp", bufs=3) as outp,
        tc.tile_pool(name="sps", bufs=1, space="PSUM") as spsp,
        tc.tile_pool(name="avps", bufs=4, space="PSUM") as avpsp,
    ):
        steps = [(p, b) for p in range(PAIRS) for b in range(BL)]
        eb_cache = {}

        def load_eb(p):
            # both heads of the pair side by side: tile layout [128, (hh n)]
            eb_sb = [None] * MT
            for mt in range(MT):
                t = ebp.tile([128, 2 * N], bf16, name="ebt", tag="ebt")
                base = eb_d[2 * p, mt]
                src = bass.AP(tensor=base.tensor, offset=base.offset,
                              ap=[base.ap[0], [MT * 128 * N, 2], base.ap[1]])
                nc.gpsimd.dma_start(t[:].rearrange("p (h n) -> p h n", h=2), src)
                eb_sb[mt] = t
            return eb_sb

        def av_chunk(st, mt):
            p, b, E, avt = st["p"], st["b"], st["E"], st["avt"]
            for hh in range(2):
                h = 2 * p + hh
                lhsT = v_sb[b][mt][:, h * 65:(h + 1) * 65]
                for nb in range(2):
                    nc.tensor.matmul(
                        avt[hh][nb][:], lhsT,
                        E[mt][:, hh * N + nb * 512:hh * N + (nb + 1) * 512],
                        start=(mt == 0), stop=(mt == MT - 1))

        def evac_step(st):
            """Denominators out + unnormalized PSUM->SBUF evacuation, with the
            copies split across ScalarE/VectorE to balance their step load."""
            p, b, avt = st["p"], st["b"], st["avt"]
            denrow = denrp.tile([65, 4 * 512], f32, name="denrow", tag="denrow")
            for hh in range(2):
                for nb in range(2):
                    q = hh * 2 + nb
                    dst = denrow[64:65, q * 512:(q + 1) * 512]
                    if hh == 0:
                        nc.scalar.copy(dst, avt[hh][nb][64:65, :])
                    else:
                        nc.vector.tensor_copy(dst, avt[hh][nb][64:65, :])
            PI = qkvout.tile([128, N], bf16, tag=f"{r}qk{b}_{2 * p}")
            projIn[b][p] = PI
            tb_t = tmpb.tile([64, N], bf16, name="tbt", tag="tbt")
            for nb in range(2):
                ns = slice(nb * 512, (nb + 1) * 512)
                nc.vector.tensor_copy(PI[0:64, ns], avt[0][nb][0:64, :])
                if nb == 0:
                    nc.scalar.copy(tb_t[:, ns], avt[1][nb][0:64, :])
                else:
                    nc.vector.tensor_copy(tb_t[:, ns], avt[1][nb][0:64, :])
            nc.sync.dma_start(PI[64:128, :], tb_t[:])
            idx = p * BL + b
            nc.sync.dma_start(den_scr[idx:idx + 1, :], denrow[64:65, :])
            denB = recp.tile([128, N], f32, tag="recp", name="denB")
            for hh in range(2):
                for nb in range(2):
                    q = hh * 2 + nb
                    src = den_scr[idx:idx + 1, q * 512:(q + 1) * 512]
                    nc.gpsimd.dma_start(
                        denB[hh * 64:(hh + 1) * 64, nb * 512:(nb + 1) * 512],
                        _bcast_ap(src, 64))
            st["denB"] = denB
            st["PI"] = PI

        def norm_step(st):
            rB = recp.tile([128, N], f32, tag="recp", name="rB")
            nc.vector.reciprocal_approx_fast(rB[:], st["denB"][:])
            nc.vector.tensor_mul(st["PI"][:], st["PI"][:], rB[:])

        def emit_proj(b):
            for tt in range(TT):
                ps = spsp.tile([128, 2 * N], f32, name="sps", tag="sps")
                for p in range(PAIRS):
                    lhsT = projIn[b][p][:, tt * 128:(tt + 1) * 128]
                    nc.tensor.matmul(ps[:, 0:512], lhsT, pw_sb[p][:, 0:512],
                                     start=(p == 0), stop=(p == PAIRS - 1))
                    nc.tensor.matmul(ps[:, 512:DIM], lhsT, pw_sb[p][:, 512:DIM],
                                     start=(p == 0), stop=(p == PAIRS - 1))
                ot = outp.tile([128, DIM], f32, name="ot", tag="ot")
                nc.vector.tensor_add(ot[:], ps[:, 0:DIM], projbB[:])
                nc.scalar.dma_start(out_d[b, tt * 128:(tt + 1) * 128, :], ot[:])

        prev = None
        pend = None
        for p, b in steps:
            if b == 0:
                eb_cache[p] = load_eb(p)
            eb_sb = eb_cache[p]
            QP = qk_sb[b][2 * p]
            KP = qk_sb[b][2 * p + 1]
            E = [None] * MT
            cur = {"p": p, "b": b, "E": E,
                   "avt": [[avpsp.tile([65, 512], f32, name="avt", tag="avt")
                            for _ in range(2)] for _ in range(2)]}
            for mt in range(MT):
                # both heads' scores of this m-chunk in one 4-bank tile so one
                # wide exp + one wide bias-multiply amortize the op overheads
                sps = spsp.tile([128, 2 * N], f32, name="sps", tag="sps")
                for hh in range(2):
                    sl = slice(hh * 64, (hh + 1) * 64)
                    lhsT = KP[sl, mt * 128:(mt + 1) * 128]
                    nc.tensor.matmul(sps[:, hh * N:hh * N + 512], lhsT,
                                     QP[sl, 0:512], start=True, stop=True)
                    nc.tensor.matmul(sps[:, hh * N + 512:(hh + 1) * N], lhsT,
                                     QP[sl, 512:N], start=True, stop=True)
                et = ep.tile([128, 2 * N], bf16, name="et", tag="et")
                nc.scalar.activation(et[:], sps[:], FP.Exp, scale=SCALE)
                nc.vector.tensor_mul(et[:], et[:], eb_sb[mt][:])
                E[mt] = et
                if prev is not None:
                    av_chunk(prev, mt)
                if mt == 3 and pend is not None:
                    norm_step(pend)
                    pend = None
            if prev is not None:
                evac_step(prev)
                pend = prev
            prev = cur
        # normalize the second-to-last step, then emit proj(b0): it fills the
        # PE while the final step's exps drain through ScalarE
        if pend is not None:
            norm_step(pend)
            pend = None
        emit_proj(0)
        # drain the pipeline
        for mt in range(MT):
            av_chunk(prev, mt)
        evac_step(prev)
        norm_step(prev)
        emit_proj(1)


# ---------------- host side ----------------

def _prep_inputs(x, ln_w, ln_b, qkv_w, qkv_b, proj_w, proj_b,
                 attn_biases, bias_idxs):
    """Fold LN affine into QKV weights; build device layouts (shared part)."""
    f64 = np.float64
    Wp = qkv_w.astype(f64) * ln_w.astype(f64)[None, :]       # [2304, 768]
    bp = qkv_b.astype(f64) + qkv_w.astype(f64) @ ln_b.astype(f64)

    def q_rows(h): return np.arange(h * 3 * KD, h * 3 * KD + KD)
    def k_rows(h): return np.arange(h * 3 * KD + KD, h * 3 * KD + 2 * KD)
    def v_rows(h): return np.arange(h * 3 * KD + 2 * KD, h * 3 * KD + 3 * KD)

    qk_order = []
    for p in range(PAIRS):
        qk_order += list(q_rows(2 * p)) + list(q_rows(2 * p + 1))
        qk_order += list(k_rows(2 * p)) + list(k_rows(2 * p + 1))
    qk_order = np.array(qk_order)
    v_order = np.concatenate([v_rows(h) for h in range(H)])

    wqk = np.ascontiguousarray(Wp[qk_order].T).astype(ml_dtypes.bfloat16)
    qkb = bp[qk_order].astype(np.float32).reshape(H, 128, 1)
    wv = np.ascontiguousarray(Wp[v_order].T).astype(ml_dtypes.bfloat16)
    vb = np.zeros((1, H * 65), np.float32)
    vbv = vb.reshape(H, 65)
    vbv[:, 0:64] = bp[v_order].astype(np.float32).reshape(H, 64)
    vbv[:, 64] = 1.0  # ones column template
    pw = np.ascontiguousarray(proj_w.T).astype(ml_dtypes.bfloat16)
    pb = proj_b.astype(np.float32).reshape(1, DIM)

    ebias = np.exp(attn_biases.astype(f64))[:, np.asarray(bias_idxs)]  # [H,n,m]
    eb = np.ascontiguousarray(ebias.transpose(0, 2, 1)).astype(
        ml_dtypes.bfloat16).reshape(H, MT, 128, N)
    return dict(wqk=wqk, qkb=qkb, wv=wv, vb=vb, pw=pw, pb=pb, eb=eb)


def _make_in_maps(x, shared):
    xt = np.ascontiguousarray(
        x.reshape(NCORES, BL, N, DIM).transpose(0, 1, 3, 2)).astype(
            ml_dtypes.bfloat16)
    return [dict(x=xt[i], **shared) for i in range(NCORES)]


_PROG = {}


def _get_program(reps=1):
    if reps not in _PROG:
        _PROG[reps] = build_program(reps)
    return _PROG[reps]


class _Runner:
    """Persistent jitted SPMD executor (mirrors bass2jax.run_bass_via_pjrt's
    multi-core branch, but the jitted callable is cached across calls)."""

    def __init__(self, nc):
        import jax
        from jax.experimental.shard_map import shard_map
        from jax.sharding import Mesh, PartitionSpec
        from concourse import mybir as _mb
        from concourse.bass2jax import _bass_exec_p, install_neuronx_cc_hook

        install_neuronx_cc_hook()
        self.jax = jax
        from concourse.bass2jax import partition_id_tensor
        part_name = (nc.partition_id_tensor.name
                     if nc.partition_id_tensor else None)
        in_names, out_names, out_avals = [], [], []
        for alloc in nc.m.functions[0].allocations:
            if not isinstance(alloc, _mb.MemoryLocationSet):
                continue
            name = alloc.memorylocations[0].name
            if alloc.kind == "ExternalInput":
                if name != part_name:
                    in_names.append(name)
            elif alloc.kind == "ExternalOutput":
                out_names.append(name)
                out_avals.append(jax.core.ShapedArray(
                    tuple(alloc.tensor_shape), _mb.dt.np(alloc.dtype)))
        self.in_names, self.out_names, self.out_avals = in_names, out_names, out_avals
        n_params, n_outs = len(in_names), len(out_names)
        bind_names = tuple(in_names + out_names
                           + ([part_name] if part_name else []))

        def _body(*args):
            operands = list(args)
            if part_name:
                operands.append(partition_id_tensor())
            return tuple(_bass_exec_p.bind(
                *operands, out_avals=tuple(out_avals), in_names=bind_names,
                out_names=tuple(out_names), lowering_input_output_aliases=(),
                sim_require_finite=True, sim_require_nnan=True, nc=nc))

        devices = jax.devices()[:NCORES]
        self.mesh = Mesh(np.asarray(devices), ("core",))
        in_specs = (PartitionSpec("core"),) * (n_params + n_outs)
        out_specs = (PartitionSpec("core"),) * n_outs
        self.sharded = jax.jit(
            shard_map(_body, mesh=self.mesh, in_specs=in_specs,
                      out_specs=out_specs, check_rep=False),
            donate_argnums=tuple(range(n_params, n_params + n_outs)),
            keep_unused=True)
        self.sharding = jax.sharding.NamedSharding(
            self.mesh, PartitionSpec("core"))

    def put_inputs(self, in_maps):
        """Concatenate per-core inputs on axis 0 and place on devices."""
        concat = [np.concatenate([np.asarray(m[n]) for m in in_maps], axis=0)
                  for n in self.in_names]
        return [self.jax.device_put(a, self.sharding) for a in concat]

    def zeros(self):
        return [self.jax.device_put(
                    np.zeros((NCORES * av.shape[0], *av.shape[1:]), av.dtype),
                    self.sharding)
                for av in self.out_avals]

    def run(self, dev_inputs, dev_zeros=None):
        if dev_zeros is None:
            dev_zeros = self.zeros()
        outs = self.sharded(*dev_inputs, *dev_zeros)
        self.jax.block_until_ready(outs)
        return outs

    def run_np(self, dev_inputs):
        outs = self.run(dev_inputs)
        res = {}
        for i, name in enumerate(self.out_names):
            a = np.asarray(outs[i])
            res[name] = a.reshape(NCORES, *self.out_avals[i].shape)
        return res


_RUNNERS = {}


def _get_runner(reps=1):
    if reps not in _RUNNERS:
        _RUNNERS[reps] = _Runner(_get_program(reps))
    return _RUNNERS[reps]


def kernel(x, ln_w, ln_b, qkv_w, qkv_b, proj_w, proj_b,
           attn_biases, bias_idxs):
    x, ln_w, ln_b, qkv_w, qkv_b, proj_w, proj_b, attn_biases, bias_idxs = (
        np.asarray(a) for a in (x, ln_w, ln_b, qkv_w, qkv_b, proj_w, proj_b,
                                attn_biases, bias_idxs))
    runner = _get_runner()
    shared = _prep_inputs(x, ln_w, ln_b, qkv_w, qkv_b, proj_w, proj_b,
                          attn_biases, bias_idxs)
    in_maps = _make_in_maps(np.asarray(x), shared)
    dev = runner.put_inputs(in_maps)
    out = runner.run_np(dev)["out"]          # [NCORES, BL, N, DIM]
    return out.reshape(B, N, DIM).astype(np.float32)
